# revision 8
# baseline (speedup 1.0000x reference)
"""Contrastive cosine-similarity MSE loss kernel for Trainium2 (8 cores).

Math (reference): scores_n = <a_n, b_n> / (||a_n|| * ||b_n||);
loss = mean((scores - labels)^2) over N=8192 rows, D=1024.

Sharding: data-parallel over rows. Core c handles rows [c*1024, (c+1)*1024).
Per core, 8 tiles of [128 rows x 1024 cols]:
  - VectorE: fused multiply+row-sum  -> dots
  - ScalarE: fused square+row-sum    -> ||a||^2, ||b||^2
Tail on [128, 8] stats: score = dots * 1/sqrt(na*nb); partial[p] =
sum_t (score - label)^2. Host sums the 8x128 partials and divides by N.
"""

import numpy as np

import concourse.bacc as bacc
import concourse.tile as tile
from concourse import mybir
from concourse.bass_utils import run_bass_kernel_spmd

N, D = 8192, 1024
N_CORES = 8
ROWS = N // N_CORES  # rows per core
P = 128  # SBUF partitions
NT = ROWS // P  # row-tiles per core

_cache = {}


def _build():
    nc = bacc.Bacc("TRN2", target_bir_lowering=False, debug=False)

    f32 = mybir.dt.float32
    a = nc.dram_tensor("a", [ROWS, D], f32, kind="ExternalInput")
    b = nc.dram_tensor("b", [ROWS, D], f32, kind="ExternalInput")
    # labels pre-transposed on host to [P, NT]: lab_t[p, t] = labels[t*P + p]
    lab = nc.dram_tensor("lab_t", [P, NT], f32, kind="ExternalInput")
    out = nc.dram_tensor("out", [P, 1], f32, kind="ExternalOutput")

    with tile.TileContext(nc) as tc:
        with (
            tc.tile_pool(name="io", bufs=3) as io_pool,
            tc.tile_pool(name="scratch", bufs=2) as scr_pool,
            tc.tile_pool(name="stats", bufs=1) as st_pool,
        ):
            dots = st_pool.tile([P, NT], f32)
            na = st_pool.tile([P, NT], f32)
            nb = st_pool.tile([P, NT], f32)
            labt = st_pool.tile([P, NT], f32)
            nc.sync.dma_start(out=labt, in_=lab[:, :])

            # Spread loads across several engines' DMA queues so more
            # descriptor rings feed the 16 DMA engines in parallel.
            a_engines = [nc.sync, nc.gpsimd, nc.sync, nc.scalar]
            b_engines = [nc.gpsimd, nc.sync, nc.scalar, nc.sync]
            for t in range(NT):
                at = io_pool.tile([P, D], f32, tag="a")
                bt = io_pool.tile([P, D], f32, tag="b")
                a_engines[t % 4].dma_start(out=at, in_=a[t * P : (t + 1) * P, :])
                b_engines[t % 4].dma_start(out=bt, in_=b[t * P : (t + 1) * P, :])

                sd = scr_pool.tile([P, D], f32, tag="sdve")
                sa = scr_pool.tile([P, D], f32, tag="sact")
                sb = scr_pool.tile([P, D], f32, tag="sdve")
                # dots[:, t] = sum_d at * bt  (VectorE, one fused pass)
                nc.vector.scalar_tensor_tensor(
                    out=sd,
                    in0=at,
                    scalar=1.0,
                    in1=bt,
                    op0=mybir.AluOpType.mult,
                    op1=mybir.AluOpType.mult,
                    accum_out=dots[:, t : t + 1],
                )
                # na[:, t] = sum_d at^2 (ScalarE); nb[:, t] = sum_d bt^2 (VectorE)
                nc.scalar.activation(
                    out=sa,
                    in_=at,
                    func=mybir.ActivationFunctionType.Square,
                    accum_out=na[:, t : t + 1],
                )
                nc.vector.scalar_tensor_tensor(
                    out=sb,
                    in0=bt,
                    scalar=1.0,
                    in1=bt,
                    op0=mybir.AluOpType.mult,
                    op1=mybir.AluOpType.mult,
                    accum_out=nb[:, t : t + 1],
                )

            # Tail on [P, NT] stats (tiny).
            prod = st_pool.tile([P, NT], f32)
            nc.vector.tensor_mul(prod, na, nb)
            nc.scalar.sqrt(prod, prod)
            rs = st_pool.tile([P, NT], f32)
            nc.vector.reciprocal(rs, prod)
            score = st_pool.tile([P, NT], f32)
            nc.vector.tensor_mul(score, dots, rs)
            diff = st_pool.tile([P, NT], f32)
            nc.vector.tensor_sub(diff, score, labt)
            sqd = st_pool.tile([P, NT], f32)
            partial = st_pool.tile([P, 1], f32)
            nc.vector.scalar_tensor_tensor(
                out=sqd,
                in0=diff,
                scalar=1.0,
                in1=diff,
                op0=mybir.AluOpType.mult,
                op1=mybir.AluOpType.mult,
                accum_out=partial,
            )
            nc.sync.dma_start(out=out[:, :], in_=partial)

    nc.compile()
    return nc


def kernel(issues_1_geb, issues_2_geb, labels):
    if "nc" not in _cache:
        _cache["nc"] = _build()
    nc = _cache["nc"]

    a = np.ascontiguousarray(issues_1_geb, dtype=np.float32)
    b = np.ascontiguousarray(issues_2_geb, dtype=np.float32)
    lab = np.ascontiguousarray(labels, dtype=np.float32)

    in_maps = []
    for c in range(N_CORES):
        sl = slice(c * ROWS, (c + 1) * ROWS)
        lab_t = np.ascontiguousarray(lab[sl].reshape(NT, P).T)
        in_maps.append(
            {
                "a": np.ascontiguousarray(a[sl]),
                "b": np.ascontiguousarray(b[sl]),
                "lab_t": lab_t,
            }
        )

    res = run_bass_kernel_spmd(nc, in_maps, core_ids=list(range(N_CORES)))
    total = np.float64(0.0)
    for r in res.results:
        total += np.float64(r["out"].sum(dtype=np.float64))
    return np.array(total / N, dtype=np.float32)
